# revision 35
# baseline (speedup 1.0000x reference)
"""Self-contained Trainium2 Bass kernel for nn_Attention (8-head self-attention).

Reference computation (per batch element b):
    xt = x[b].reshape(C, N).T            # (N, C),  N = H*W = 1024
    q  = xt @ Wq                         # (N, 512)
    k, v = split(xt @ Wkv)               # (N, 512) each
    per head h (d=64): sim = q_h k_h^T / 8 ; P = softmax(sim) ; o_h = P v_h
    out[b] = concat_h(o_h) @ Wo + bo     # (N, C)

Sharding: pure data parallel -- core b computes batch element b (8 cores, 8
batch elements, no collectives).

Layout strategy (keeps every matmul contraction dim on partitions, zero
on-chip transposes):
  - x[b] is used as (C, N): already the transpose of xt.
  - qT, kT are produced in (inner, N) layout; V in (N, inner) layout with an
    extra ones-column per head so the P@V matmul also emits the softmax
    denominators (M = 64+1 = 65).
  - Heads are processed in PAIRS (2m, 2m+1). Head 2m lives on partitions
    0:64 of qT/kT[:, m, :], head 2m+1 on 64:128, so their K=64 sim matmuls
    occupy disjoint PE row groups and execute CONCURRENTLY (row tiling).
  - Sim tiles are QUERY-HALVED: [128 keys, 512 queries] = one PSUM bank,
    so each exp consumer gets a double-buffered chain (psA/psB bufs=2) and
    the sim -> exp -> sim PSUM-reuse chain never serializes the slot.
  - exp is split across two engines: ScalarE runs the real activation on
    two of the four (head, qhalf) tiles per key-tile step, VectorE runs a
    Schraudolph-style fast exp (affine in f32, convert to int16 = the bf16
    bit pattern of exp, bitcast back to bf16) on the other two. Fast-exp
    max rel err ~3% pre-softmax; the systematic part cancels in the
    normalization; end-to-end contribution ~3e-3.
  - attn@v: one F=512 matmul per (head, key tile, query half) into a
    [65, N] PSUM accumulator (matmul output cannot cross a PSUM bank).
  - Pipelining: attn@v for key tile jt runs at slot jt+2; the previous
    pair's epilogue occupies the next pair's early slots: ov drained to
    bf16 SBUF in 512-halves (hh=0 on ScalarE, hh=1 on VectorE), the
    denominator row scattered to [128, 8] by one SBUF->SBUF DMA, a
    64-cycle reciprocal, a DRAM-bounce partition broadcast, and the
    normalize multiply on GpSimd (kept off the exp engines' queues).
    The last pair normalizes on VectorE and the output projection runs
    kk-major in two halves so only the final accumulation step waits on
    the epilogue chain; warmup matmuls bridge the latency bubble so the
    PE clock gate stays at 8/8.
  - Inputs are pre-cast to bf16 on the host (halves DMA bytes, removes
    all on-device casts) and loaded over three DMA queues in parallel.
  - HAM warmup: dummy matmuls paced by the arriving input chunks keep the
    PE activity monitor from idling the clock to 1.2 GHz before/through
    the compute phases.
  - PSUM budget: 4 sim banks + two [65, N] attn@v accumulators (2 banks
    each, double buffered across pairs) = 8 banks.
"""

import numpy as np

import concourse.bass as bass
import concourse.mybir as mybir
import concourse.tile as tile
from concourse import bacc

B, C, N = 8, 512, 1024
HEADS, D = 8, 64
INNER = HEADS * D  # 512
SCALE = D ** -0.5
P = 128
CT = C // P       # 4  k-tiles over C
MT = INNER // P   # 4  partition-tiles over inner
JT = N // P       # 8  key tiles
NT = N // P       # 8  output row tiles
NB = N // 512     # 2  free-dim blocks of 512 over N
PAIRS = HEADS // 2

F32 = mybir.dt.float32
BF16 = mybir.dt.bfloat16
I16 = mybir.dt.int16
EXP = mybir.ActivationFunctionType.Exp
COPY = mybir.ActivationFunctionType.Copy
MULT = mybir.AluOpType.mult
ADD = mybir.AluOpType.add

# fast-exp constants: bf16 bits of exp(s * SCALE) ~= int16(FEXP_A * s + FEXP_B)
FEXP_A = 128.0 * 1.4426950408889634 * SCALE
FEXP_B = 16250.65

AV_F = 512  # attn@v free size (matmul output cannot cross a PSUM bank)


def build_nc(debug=False):
    nc = bacc.Bacc(
        "TRN2", target_bir_lowering=False, debug=debug, num_devices=B
    )
    # inputs are pre-cast to bf16 on the host: halves the load bytes and
    # removes every on-device cast
    x_d = nc.dram_tensor("x", [C, N], BF16, kind="ExternalInput")
    wq_d = nc.dram_tensor("Wq", [C, INNER], BF16, kind="ExternalInput")
    wkv_d = nc.dram_tensor("Wkv", [C, 2 * INNER], BF16, kind="ExternalInput")
    wo_d = nc.dram_tensor("Wo", [INNER, C], BF16, kind="ExternalInput")
    bo_d = nc.dram_tensor("bo", [C], F32, kind="ExternalInput")
    out_d = nc.dram_tensor("out", [N, C], F32, kind="ExternalOutput")

    with tile.TileContext(nc) as tc:
        with (
            tc.tile_pool(name="persist", bufs=1) as persist,
            tc.tile_pool(name="etp", bufs=2) as etp,
            tc.tile_pool(name="small", bufs=4) as small,
            tc.tile_pool(name="dramp", bufs=2, space="DRAM") as dramp,
            tc.tile_pool(name="psA", bufs=2, space="PSUM") as psA,
            tc.tile_pool(name="psB", bufs=2, space="PSUM") as psB,
            tc.tile_pool(name="psO", bufs=2, space="PSUM") as psO,
        ):
            # ---------------- load inputs (pre-cast bf16) ----------------
            # spread across three DMA queues (~146 GB/s per queue): sync
            # gets x0,x1 + wkv2,3; scalar gets x2,x3; gpsimd gets wq, wkv0,1,
            # wo, bo.
            x_b = persist.tile([P, CT, N], BF16)
            x_dv = x_d[:].rearrange("(a p) n -> p a n", p=P)
            wq_b = persist.tile([P, CT, INNER], BF16)
            wq_dv = wq_d[:].rearrange("(a p) m -> p a m", p=P)
            wkv_b = persist.tile([P, CT, 2 * INNER], BF16)
            wkv_dv = wkv_d[:].rearrange("(a p) m -> p a m", p=P)

            # HAM warmup scratch: PE work paced by the arriving input chunks
            # keeps the clock gate at 8/8 through the projection phase.
            warm = psO.tile([D + 1, N], F32, tag="ov", name="warm")

            def warm_mms(src, n):
                for r in range(n):
                    nc.tensor.matmul(
                        warm[0:D, 0:512],
                        lhsT=src[:, (r % 4) * D:(r % 4) * D + D],
                        rhs=src[:, 0:512],
                        start=True, stop=True,
                        skip_group_check=True,
                    )

            bo_bc = persist.tile([P, C], F32)
            bo_ap = bo_d[:]
            for a in range(2):
                nc.sync.dma_start(out=x_b[:, a, :], in_=x_dv[:, a, :])
                nc.scalar.dma_start(
                    out=x_b[:, a + 2, :], in_=x_dv[:, a + 2, :])
                nc.gpsimd.dma_start(out=wq_b[:, a, :], in_=wq_dv[:, a, :])
                nc.gpsimd.dma_start(
                    out=wq_b[:, a + 2, :], in_=wq_dv[:, a + 2, :])
            for a in range(2):
                nc.gpsimd.dma_start(out=wkv_b[:, a, :], in_=wkv_dv[:, a, :])
                nc.sync.dma_start(
                    out=wkv_b[:, a + 2, :], in_=wkv_dv[:, a + 2, :])
            wo_b = persist.tile([P, MT, C], BF16)
            nc.scalar.dma_start(
                out=wo_b, in_=wo_d[:].rearrange("(a p) m -> p a m", p=P))
            nc.gpsimd.dma_start(
                out=bo_bc,
                in_=bass.AP(tensor=bo_ap.tensor, offset=bo_ap.offset,
                            ap=[[0, P], [1, C]]),
            )
            # warmup matmuls paced by the arriving x chunks
            for a in range(CT):
                warm_mms(x_b[:, a, :], 12 if a == 0 else 3)

            zb = persist.tile([P, 1], F32)
            nc.vector.memset(zb, 0.0)

            # ---------------- projections ----------------
            qT = persist.tile([P, MT, N], BF16)
            kT = persist.tile([P, MT, N], BF16)
            v_ext = persist.tile([P, JT, HEADS, D + 1], BF16)
            nc.vector.memset(v_ext[:, :, :, D], 1.0)

            def kq_proj2(mts, dst, w_b):
                """Two projection columns, chunk-major: each arriving x/W
                chunk is consumed across all four in-flight accumulators so
                the PE rides the DMA/cast pipeline instead of stalling on
                the last chunk of the first column."""
                ps = {}
                for mt in mts:
                    for ib in range(NB):
                        ps[(mt, ib)] = (psA if ib == 0 else psB).tile(
                            [P, 512], F32, tag="sim", name="pj")
                for a in range(CT):
                    for mt in mts:
                        for ib in range(NB):
                            nc.tensor.matmul(
                                ps[(mt, ib)],
                                lhsT=w_b[:, a, mt * P:(mt + 1) * P],
                                rhs=x_b[:, a, ib * 512:(ib + 1) * 512],
                                start=(a == 0),
                                stop=(a == CT - 1),
                            )
                for mt in mts:
                    for ib in range(NB):
                        if dst is kT:
                            nc.scalar.activation(
                                out=dst[:, mt, ib * 512:(ib + 1) * 512],
                                in_=ps[(mt, ib)], func=COPY)
                        else:
                            nc.vector.tensor_copy(
                                out=dst[:, mt, ib * 512:(ib + 1) * 512],
                                in_=ps[(mt, ib)])

            def v_proj2(jts):
                # V in normal layout (token j on partitions), per head with an
                # extra ones column: v_ext[:, jt, h, 0:64] = V, [..., 64] = 1
                ps = {}
                for i, jt in enumerate(jts):
                    ps[jt] = (psA if i % 2 == 0 else psB).tile(
                        [P, 512], F32, tag="sim", name="pv")
                for a in range(CT):
                    for jt in jts:
                        nc.tensor.matmul(
                            ps[jt],
                            lhsT=x_b[:, a, jt * P:(jt + 1) * P],
                            rhs=wkv_b[:, a, INNER:2 * INNER],
                            start=(a == 0),
                            stop=(a == CT - 1),
                        )
                for jt in jts:
                    nc.vector.tensor_copy(
                        out=v_ext[:, jt, :, 0:D],
                        in_=ps[jt].rearrange("p (h d) -> p h d", h=HEADS),
                    )

            kq_proj2((0, 1), qT, wq_b)
            kq_proj2((2, 3), qT, wq_b)
            kq_proj2((0, 1), kT, wkv_b)
            kq_proj2((2, 3), kT, wkv_b)
            for j0 in range(0, JT, 2):
                v_proj2((j0, j0 + 1))

            # ---------------- attention (pair-pipelined) ----------------
            oTs = []
            for m in range(PAIRS):
                oT_m = persist.tile([P, N], BF16, tag=f"oT{m}")
                oTs.append(oT_m)

            et_tiles = {}
            ov_tiles = {}

            def emit_sim(m, jt):
                """Row-tiled query-halved sims. Four [128 keys, 512 q] tiles:
                (h0,q0)+(h1,q1) feed ScalarE exp, (h1,q0)+(h0,q1) feed the
                VectorE fast exp. Tiles of a pair use disjoint PE row groups
                so they run concurrently."""
                et = et_tiles[m]
                ea = []
                for pool, pairs_ in (
                    (psA, ((0, 0), (1, 1))),
                    (psB, ((1, 0), (0, 1))),
                ):
                    for hh, qh in pairs_:
                        st = pool.tile([P, 512], F32, tag="sim",
                                       name=f"st{hh}{qh}")
                        hp = hh * D
                        nc.tensor.matmul(
                            st,
                            lhsT=kT[hp:hp + D, m, jt * P:(jt + 1) * P],
                            rhs=qT[hp:hp + D, m, qh * 512:(qh + 1) * 512],
                            start=True, stop=True,
                        )
                        ea.append((hh, qh, st))
                # each pool's two tiles drain on DIFFERENT engines so both
                # buffers release concurrently: ScalarE gets ea[0] (psA) +
                # ea[2] (psB), VectorE gets ea[1] (psA) + ea[3] (psB).
                for hh, qh, st in (ea[0], ea[2]):
                    nc.scalar.activation(
                        out=et[:, hh, jt, qh * 512:(qh + 1) * 512],
                        in_=st, func=EXP, bias=zb, scale=SCALE)
                for hh, qh, st in (ea[1], ea[3]):
                    nc.vector.tensor_scalar(
                        et[:, hh, jt, qh * 512:(qh + 1) * 512].bitcast(I16),
                        st,
                        FEXP_A,
                        FEXP_B,
                        MULT,
                        ADD,
                    )

            def emit_av(m, jt):
                """attn@v for key tile jt of pair m (one F=1024 matmul per
                head; output [65, 1024] spans 2 PSUM banks)."""
                et = et_tiles[m]
                for hh in range(2):
                    ov = ov_tiles[(m, hh)]
                    if AV_F == 1024:
                        nc.tensor.matmul(
                            ov,
                            lhsT=v_ext[:, jt, 2 * m + hh, :],
                            rhs=et[:, hh, jt, :],
                            start=(jt == 0),
                            stop=(jt == JT - 1),
                        )
                    else:
                        for ib in range(NB):
                            sl = slice(ib * 512, (ib + 1) * 512)
                            nc.tensor.matmul(
                                ov[:, sl],
                                lhsT=v_ext[:, jt, 2 * m + hh, :],
                                rhs=et[:, hh, jt, sl],
                                start=(jt == 0),
                                stop=(jt == JT - 1),
                            )

            def emit_drain(m, hh):
                """Drain ov (o rows + denom row) to bf16 SBUF in two halves
                that fit the slack between exps. hh=0 goes on ScalarE, hh=1
                on VectorE so the per-pair drain cost splits across both."""
                ov = ov_tiles.pop((m, hh))
                ov_sb = small.tile([D + 1, N], BF16, tag="ovsb")
                for ib in range(NB):
                    sl = slice(ib * 512, (ib + 1) * 512)
                    if hh == 0:
                        nc.scalar.activation(
                            out=ov_sb[:, sl], in_=ov[:, sl], func=COPY)
                    else:
                        nc.vector.tensor_copy(out=ov_sb[:, sl], in_=ov[:, sl])
                return ov_sb

            def emit_recip(ov_sb):
                """Denominator reciprocal: scatter the row to [128, 8] (one
                SBUF->SBUF DMA), 64-cycle reciprocal, bounce back through
                DRAM as a [64, N] partition broadcast."""
                st2 = small.tile([P, NT], BF16, tag="st2")
                nc.sync.dma_start(out=st2, in_=ov_sb[D:D + 1, :])
                rst2 = small.tile([P, NT], BF16, tag="rst2")
                with nc.allow_low_precision(
                        reason="softmax denom reciprocal in bf16; "
                        "~0.4% rel err, within the 2e-2 budget"):
                    nc.vector.reciprocal(rst2, st2)
                rsd = dramp.tile([N], BF16, tag="rsd")
                nc.sync.dma_start(
                    out=rsd.rearrange("(p k) -> p k", k=NT), in_=rst2)
                rep = small.tile([D, N], BF16, tag="rep")
                rsd_ap = rsd[:]
                nc.sync.dma_start(
                    out=rep,
                    in_=bass.AP(tensor=rsd_ap.tensor,
                                offset=rsd_ap.offset,
                                ap=[[0, D], [1, N]]),
                )
                return rep

            def emit_norm(m, hh, ov_sb, rep, eng=None):
                hp = hh * D
                eng = eng or nc.gpsimd
                eng.tensor_mul(
                    oTs[m][hp:hp + D, :], ov_sb[0:D, :], rep)

            # deferred-work schedule per slot (m, jt):
            #   jt>=2  : attn@v (m, jt-2)
            #   jt=0,1 : attn@v (m-1, 6) / (m-1, 7)
            #   jt=1,2 : denominator reciprocals of pair m-1
            #   jt=2,3 : drains of pair m-1 (hh=0,1)
            #   jt=4,5 : norms of pair m-1
            pending_rep = {}
            pending_ovsb = {}
            for m in range(PAIRS):
                et_tiles[m] = etp.tile([P, 2, JT, N], BF16, tag="et",
                                       name=f"et{m}")
                for hh in range(2):
                    ov_tiles[(m, hh)] = psO.tile([D + 1, N], F32, tag="ov",
                                                 name=f"ov{m}_{hh}")
                for jt in range(JT):
                    emit_sim(m, jt)
                    if jt >= 2:
                        emit_av(m, jt - 2)
                    elif m > 0:
                        emit_av(m - 1, JT - 2 + jt)
                    if m > 0:
                        if jt in (1, 2):
                            hh = jt - 1
                            ov_sb = emit_drain(m - 1, hh)
                            pending_ovsb[hh] = ov_sb
                            pending_rep[hh] = emit_recip(ov_sb)
                        elif jt in (4, 5):
                            hh = jt - 4
                            emit_norm(m - 1, hh, pending_ovsb.pop(hh),
                                      pending_rep.pop(hh))
            # tail: last pair's remaining chunks + epilogue, overlapped
            # with the first output-projection accumulations (kk=0..2 do not
            # need the still-draining pair-3 heads).
            m = PAIRS - 1
            emit_av(m, JT - 2)
            emit_av(m, JT - 1)
            d0 = emit_drain(m, 0)
            rep0 = emit_recip(d0)
            d1 = emit_drain(m, 1)
            rep1 = emit_recip(d1)
            op_tiles = {}

            def op_head(its):
                for it in its:
                    op_tiles[it] = (psA if it % 2 == 0 else psB).tile(
                        [P, 512], F32, tag="sim", name="op")
                for kk in range(MT - 1):
                    for it in its:
                        nc.tensor.matmul(
                            op_tiles[it],
                            lhsT=oTs[kk][:, it * P:(it + 1) * P],
                            rhs=wo_b[:, kk, :],
                            start=(kk == 0),
                            stop=False,
                        )

            def op_tail(its):
                kk = MT - 1
                for it in its:
                    pf = op_tiles.pop(it)
                    nc.tensor.matmul(
                        pf,
                        lhsT=oTs[kk][:, it * P:(it + 1) * P],
                        rhs=wo_b[:, kk, :],
                        start=False,
                        stop=True,
                    )
                    fin = small.tile([P, C], F32, tag="fin")
                    nc.vector.tensor_add(fin, pf, bo_bc)
                    (nc.sync if it % 2 == 0 else nc.scalar).dma_start(
                        out=out_d[it * P:(it + 1) * P, :], in_=fin)

            op_head(range(0, 4))
            # keep the PE clock warm across the epilogue-latency bubble
            warm2 = psO.tile([D + 1, N], F32, tag="ov", name="warm2")
            for r in range(20):
                nc.tensor.matmul(
                    warm2[0:D, 0:512],
                    lhsT=x_b[:, r % 4, 0:D],
                    rhs=x_b[:, r % 4, 0:512],
                    start=True, stop=True,
                    skip_group_check=True,
                )
            emit_norm(m, 0, d0, rep0, eng=nc.vector)
            emit_norm(m, 1, d1, rep1, eng=nc.vector)
            op_tail(range(0, 4))
            op_head(range(4, NT))
            op_tail(range(4, NT))

    return nc


def kernel(x, Wq, Wkv, Wo, bo):
    from concourse.bass_utils import run_bass_kernel_spmd

    nc = build_nc()
    nc.compile()
    import ml_dtypes

    bf = ml_dtypes.bfloat16
    x = np.asarray(x)
    xs = np.ascontiguousarray(x.reshape(B, C, N)).astype(bf)
    wq_b = np.asarray(Wq).astype(bf)
    wkv_b = np.asarray(Wkv).astype(bf)
    wo_b = np.asarray(Wo).astype(bf)
    bo_f = np.asarray(bo, dtype=np.float32)
    in_maps = [
        {
            "x": xs[b],
            "Wq": wq_b,
            "Wkv": wkv_b,
            "Wo": wo_b,
            "bo": bo_f,
        }
        for b in range(B)
    ]
    res = run_bass_kernel_spmd(nc, in_maps, list(range(B)))
    return np.stack([res.results[b]["out"] for b in range(B)], axis=0)
